# revision 28
# baseline (speedup 1.0000x reference)
"""Dense masked attention (B=16, S=2048, D=256) on 8 trn2 NeuronCores.

Data-parallel over batch: each core handles 2 batches, mask replicated.
Returns (output [16,2048,256] f32, attn_weights [16,2048,2048] f32),
matching the reference's (output, attn_weights) tuple.

Per-core design (per batch b, per 128-row tile rt, in two 1024-col halves
double-buffered through PSUM so exp overlaps the next half's matmuls):
  mask bias  : (mask-1)*60000 as fp16, built once on GpSimd; seeded into
               PSUM via an identity matmul so masked scores sit at -60000
               and exp underflows them to exactly 0.0
  scores     : fp16 QK^T matmuls on TensorE accumulating into the seeded
               f32 PSUM (q/k are loaded f32, cast to fp16, transposed to
               d-major via PE transpose once per batch)
  p16        : exp(scores/sqrt(D)) on ScalarE, PSUM->SBUF fp16, with the
               row-sum accumulated for free via accum_out
  inv        : 1/rowsum on VectorE
  pT         : PE transpose of p16 (fp16, PSUM) + VectorE eviction
  out        : pT.T @ v16 on TensorE (16-chunk accumulation), scaled by
               inv on VectorE
  attn       : p16 * inv -> f32 (h0 on ScalarE, h1 on VectorE after the
               pT evictions so the DVE never stalls the AV matmuls)
Wall time is bounded below by ~176 us of DMA (67 MB at ~384 GB/s/core)
and ~205 us of TensorE; measured ~264 us per kernel execution.
"""

import sys

sys.path.insert(0, "/opt/trn_rl_repo")

from contextlib import ExitStack

import numpy as np

import concourse.bacc as bacc
import concourse.bass as bass
import concourse.tile as tile
from concourse import mybir
from concourse.bass_utils import run_bass_kernel_spmd
from concourse.masks import make_identity

N_CORES = 8
B, S, D = 16, 2048, 256
B_LOC = B // N_CORES  # batches per core
SCALE = 1.0 / float(np.sqrt(D))

F32 = mybir.dt.float32
F16 = mybir.dt.float16
I32 = mybir.dt.int32

P = 128  # partitions
N_RT = S // P  # row tiles per batch (16)
N_CB = S // 512  # 512-wide column blocks in scores (4)
N_DC = D // P  # contraction chunks for QK (2)
N_KC = S // P  # kcol chunks for AV (16)


def build_kernel(b_loc=B_LOC, s=S, d=D):
    n_rt = s // P
    n_cb = s // 512
    n_dc = d // P
    n_kc = s // P

    nc = bacc.Bacc("TRN2", target_bir_lowering=False, debug=False)
    q_d = nc.declare_dram_parameter("q", [b_loc, s, d], F32, isOutput=False)
    k_d = nc.declare_dram_parameter("k", [b_loc, s, d], F32, isOutput=False)
    v_d = nc.declare_dram_parameter("v", [b_loc, s, d], F32, isOutput=False)
    m_d = nc.declare_dram_parameter("mask", [s, s], I32, isOutput=False)
    o_d = nc.declare_dram_parameter("out", [b_loc, s, d], F32, isOutput=True)
    a_d = nc.declare_dram_parameter("attn", [b_loc, s, s], F32, isOutput=True)

    with ExitStack() as ctx:
        tc = ctx.enter_context(tile.TileContext(nc))

        consts = ctx.enter_context(tc.tile_pool(name="consts", bufs=1))
        qkv = ctx.enter_context(tc.tile_pool(name="qkv", bufs=2))
        stage = ctx.enter_context(tc.tile_pool(name="stage", bufs=4))
        mstage = ctx.enter_context(tc.tile_pool(name="mstage", bufs=2))
        work = ctx.enter_context(tc.tile_pool(name="work", bufs=2))
        attnp = ctx.enter_context(tc.tile_pool(name="attnp", bufs=3))
        psum = ctx.enter_context(tc.tile_pool(name="psum", bufs=1, space="PSUM"))

        ident16 = consts.tile([P, P], F16)
        make_identity(nc, ident16)

        # ---- mask tiles are streamed during batch 0 (see load_mask_tile):
        # cast int32 -> fp16 bias (mask-1)*60000; masked scores then get
        # -60000 added pre-exp so exp underflows to exactly 0.0
        mask16 = consts.tile([P, n_rt, s], F16)

        def load_mask_tile(rt):
            mstg = mstage.tile([P, s], I32, tag="mstg", bufs=3)
            nc.sync.dma_start(out=mstg, in_=m_d[rt * P : (rt + 1) * P, :])
            nc.gpsimd.tensor_scalar(
                out=mask16[:, rt, :],
                in0=mstg,
                scalar1=-1.0,
                scalar2=60000.0,
                op0=mybir.AluOpType.add,
                op1=mybir.AluOpType.mult,
            )

        def prep_batch(b):
            """Load q,k,v for batch b; build qT16/kT16 (d-on-partitions) + v16."""
            qT = qkv.tile([P, n_dc, s], F16, tag="qT")
            kT = qkv.tile([P, n_dc, s], F16, tag="kT")
            v16 = qkv.tile([P, n_kc, d], F16, tag="v16")
            # k first: row-tile 0's QK needs all of kT but only qT[:, :, 0:P]
            for src, dstT in ((k_d, kT), (q_d, qT)):
                for g in range(n_rt // 2):  # 2 row-tiles per DMA
                    stg = stage.tile([P, 2, d], F32, tag="stg")
                    nc.sync.dma_start(
                        out=stg,
                        in_=src[b, g * 2 * P : (g + 1) * 2 * P, :].rearrange(
                            "(t p) d -> p t d", p=P
                        ),
                    )
                    c16 = stage.tile([P, 2, d], F16, tag="c16")
                    nc.vector.tensor_copy(c16, stg)
                    for t in range(2):
                        rt = g * 2 + t
                        # transpose [128q x 128d] -> [128d x 128q] per d-chunk
                        pprep = psum.tile([P, n_dc, P], F16, tag="ps_t", bufs=2)
                        for dc in range(n_dc):
                            nc.tensor.transpose(
                                pprep[:, dc, :],
                                c16[:, t, dc * P : (dc + 1) * P],
                                ident16,
                            )
                        nc.vector.tensor_copy(
                            dstT[:, :, rt * P : (rt + 1) * P], pprep
                        )
            for g in range(n_kc // 2):
                stg = stage.tile([P, 2, d], F32, tag="stg")
                nc.sync.dma_start(
                    out=stg,
                    in_=v_d[b, g * 2 * P : (g + 1) * 2 * P, :].rearrange(
                        "(t p) d -> p t d", p=P
                    ),
                )
                nc.vector.tensor_copy(v16[:, g * 2 : (g + 1) * 2, :], stg)
            return qT, kT, v16

        for b in range(b_loc):
            qT, kT, v16 = prep_batch(b)
            if b == 0:
                load_mask_tile(0)
                if n_rt > 1:
                    load_mask_tile(1)
            for rt in range(n_rt):
                if b == 0 and rt + 2 < n_rt:
                    load_mask_tile(rt + 2)
                # ---- mask-bias seed + QK^T + exp, pipelined in half-tiles ----
                n_h = max(1, s // 1024)
                H = s // n_h
                p16 = work.tile([P, s], F16, tag="p16", bufs=3)
                rsums = work.tile([P, n_h], F32, tag="rsums")
                for h in range(n_h):
                    ps_h = psum.tile([P, H], F32, tag="ps_s", bufs=2)
                    for cb in range(H // 512):
                        base = h * H + cb * 512
                        nc.tensor.matmul(
                            ps_h[:, cb * 512 : (cb + 1) * 512],
                            lhsT=ident16,
                            rhs=mask16[:, rt, base : base + 512],
                            start=True,
                            stop=False,
                        )
                    for dc in range(n_dc):
                        for cb in range(H // 512):
                            base = h * H + cb * 512
                            nc.tensor.matmul(
                                ps_h[:, cb * 512 : (cb + 1) * 512],
                                lhsT=qT[:, dc, rt * P : (rt + 1) * P],
                                rhs=kT[:, dc, base : base + 512],
                                start=False,
                                stop=(dc == n_dc - 1),
                            )
                    # exp + partial row-sum (ScalarE, fused PSUM eviction)
                    nc.scalar.activation(
                        p16[:, h * H : (h + 1) * H],
                        ps_h,
                        mybir.ActivationFunctionType.Exp,
                        scale=SCALE,
                        accum_out=rsums[:, h : h + 1],
                    )
                inv = work.tile([P, 1], F32, tag="inv")
                if n_h == 2:
                    rowsum = work.tile([P, 1], F32, tag="rowsum")
                    nc.vector.tensor_add(rowsum, rsums[:, 0:1], rsums[:, 1:2])
                else:
                    rowsum = rsums[:, 0:1]
                nc.vector.reciprocal(inv, rowsum)
                # ---- transpose pm16 via identity matmuls ----
                pT16 = work.tile([P, s], F16, tag="pT16", bufs=3)
                for h in range(n_h):
                    ps_t = psum.tile([P, H], F16, tag="ps_t", bufs=2)
                    for c in range(H // P):
                        cc = h * (H // P) + c
                        nc.tensor.transpose(
                            ps_t[:, c * P : (c + 1) * P],
                            p16[:, cc * P : (cc + 1) * P],
                            ident16,
                        )
                    nc.vector.tensor_copy(pT16[:, h * H : (h + 1) * H], ps_t)
                # ---- attn @ v (PE, accumulate over kcol chunks) ----
                ps_o = psum.tile([P, d], F32, tag="ps_o", bufs=2)
                for c in range(n_kc):
                    nc.tensor.matmul(
                        ps_o,
                        lhsT=pT16[:, c * P : (c + 1) * P],
                        rhs=v16[:, c, :],
                        start=(c == 0),
                        stop=(c == n_kc - 1),
                    )
                # ---- attn weights out — normalize h0 on ScalarE, h1 on
                # VectorE (after pT evicts), so the DVE chain never delays
                # the AV matmuls ----
                attn_f32 = attnp.tile([P, s], F32, tag="attn")
                nc.scalar.activation(
                    attn_f32[:, 0:H],
                    p16[:, 0:H],
                    mybir.ActivationFunctionType.Copy,
                    bias=0.0,
                    scale=inv,
                )
                nc.vector.tensor_scalar(
                    out=attn_f32[:, H:s],
                    in0=p16[:, H:s],
                    scalar1=inv,
                    scalar2=None,
                    op0=mybir.AluOpType.mult,
                )
                nc.sync.dma_start(
                    out=a_d[b, rt * P : (rt + 1) * P, :], in_=attn_f32
                )
                # ---- scale by inv (ScalarE) + DMA out ----
                o_f32 = work.tile([P, d], F32, tag="o_f32")
                nc.vector.tensor_scalar(
                    out=o_f32, in0=ps_o, scalar1=inv, scalar2=None,
                    op0=mybir.AluOpType.mult,
                )
                nc.sync.dma_start(out=o_d[b, rt * P : (rt + 1) * P, :], in_=o_f32)

    nc.finalize()
    return nc


_NC_CACHE = {}


def _get_nc():
    if "nc" not in _NC_CACHE:
        _NC_CACHE["nc"] = build_kernel()
    return _NC_CACHE["nc"]


def _ensure_ntff_hook():
    """The container's antenv stub lacks axon_hooks, so boot never registers
    the NTFF profile hook. Inject the module and register it ourselves."""
    import sys as _sys
    import types as _types

    if "antenv.axon_hooks" not in _sys.modules:
        mod = _types.ModuleType("antenv.axon_hooks")
        mod._hook = None

        def set_axon_ntff_profile_hook(h):
            mod._hook = h

        def get_axon_ntff_profile_hook():
            return mod._hook

        mod.set_axon_ntff_profile_hook = set_axon_ntff_profile_hook
        mod.get_axon_ntff_profile_hook = get_axon_ntff_profile_hook
        _sys.modules["antenv.axon_hooks"] = mod
        import antenv

        antenv.axon_hooks = mod
    mod = _sys.modules["antenv.axon_hooks"]
    if mod.get_axon_ntff_profile_hook() is None:
        from trn_agent_boot.trn_boot import _ntff_profile_via_ctypes

        mod.set_axon_ntff_profile_hook(
            _ntff_profile_via_ctypes("/opt/axon/libaxon_pjrt.so")
        )


def kernel(q, k, v, mask, trace=False):
    q = np.ascontiguousarray(np.asarray(q, dtype=np.float32))
    k = np.ascontiguousarray(np.asarray(k, dtype=np.float32))
    v = np.ascontiguousarray(np.asarray(v, dtype=np.float32))
    mask_full = np.ascontiguousarray(np.asarray(mask, dtype=np.int32)).reshape(S, S)

    if trace:
        _ensure_ntff_hook()
    nc = _get_nc()
    in_maps = []
    for c in range(N_CORES):
        sl = slice(c * B_LOC, (c + 1) * B_LOC)
        in_maps.append(
            {"q": q[sl], "k": k[sl], "v": v[sl], "mask": mask_full}
        )
    res = run_bass_kernel_spmd(nc, in_maps, list(range(N_CORES)), trace=trace)
    out = np.concatenate([res.results[c]["out"] for c in range(N_CORES)], axis=0)
    attn = np.concatenate([res.results[c]["attn"] for c in range(N_CORES)], axis=0)
    if trace:
        return (out, attn), res
    return out, attn


# revision 30
# speedup vs baseline: 1.0493x; 1.0493x over previous
"""Dense masked attention (B=16, S=2048, D=256) on 8 trn2 NeuronCores.

Data-parallel over batch: each core handles 2 batches, mask replicated.
Returns (output [16,2048,256] f32, attn_weights [16,2048,2048] f32),
matching the reference's (output, attn_weights) tuple.

Per-core design (per batch b, per 128-row tile rt, in two 1024-col halves
double-buffered through PSUM so exp overlaps the next half's matmuls):
  mask bias  : (mask-1)*60000 as fp16, built once on GpSimd; seeded into
               PSUM via an identity matmul so masked scores sit at -60000
               and exp underflows them to exactly 0.0
  scores     : fp16 QK^T matmuls on TensorE accumulating into the seeded
               f32 PSUM (q/k are loaded f32, cast to fp16, transposed to
               d-major via PE transpose once per batch)
  p16        : exp(scores/sqrt(D)) on ScalarE, PSUM->SBUF fp16, with the
               row-sum accumulated for free via accum_out
  inv        : 1/rowsum on VectorE
  pT         : PE transpose of p16 (fp16, PSUM) + VectorE eviction
  out        : pT.T @ v16 on TensorE (16-chunk accumulation), scaled by
               inv on VectorE
  attn       : p16 * inv -> f32 (h0 on ScalarE, h1 on VectorE after the
               pT evictions so the DVE never stalls the AV matmuls)
Wall time is bounded below by ~176 us of DMA (67 MB at ~384 GB/s/core)
and ~205 us of TensorE; measured ~264 us per kernel execution.
"""

import sys

sys.path.insert(0, "/opt/trn_rl_repo")

from contextlib import ExitStack

import numpy as np

import concourse.bacc as bacc
import concourse.bass as bass
import concourse.tile as tile
from concourse import mybir
from concourse.bass_utils import run_bass_kernel_spmd
from concourse.masks import make_identity

N_CORES = 8
B, S, D = 16, 2048, 256
B_LOC = B // N_CORES  # batches per core
SCALE = 1.0 / float(np.sqrt(D))

F32 = mybir.dt.float32
F16 = mybir.dt.float16
I32 = mybir.dt.int32

P = 128  # partitions
N_RT = S // P  # row tiles per batch (16)
N_CB = S // 512  # 512-wide column blocks in scores (4)
N_DC = D // P  # contraction chunks for QK (2)
N_KC = S // P  # kcol chunks for AV (16)


def build_kernel(b_loc=B_LOC, s=S, d=D):
    n_rt = s // P
    n_cb = s // 512
    n_dc = d // P
    n_kc = s // P

    nc = bacc.Bacc("TRN2", target_bir_lowering=False, debug=False)
    q_d = nc.declare_dram_parameter("q", [b_loc, s, d], F32, isOutput=False)
    k_d = nc.declare_dram_parameter("k", [b_loc, s, d], F32, isOutput=False)
    v_d = nc.declare_dram_parameter("v", [b_loc, s, d], F32, isOutput=False)
    m_d = nc.declare_dram_parameter("mask", [s, s], I32, isOutput=False)
    o_d = nc.declare_dram_parameter("out", [b_loc, s, d], F32, isOutput=True)
    a_d = nc.declare_dram_parameter("attn", [b_loc, s, s], F32, isOutput=True)

    with ExitStack() as ctx:
        tc = ctx.enter_context(tile.TileContext(nc))

        consts = ctx.enter_context(tc.tile_pool(name="consts", bufs=1))
        qkv = ctx.enter_context(tc.tile_pool(name="qkv", bufs=2))
        stage = ctx.enter_context(tc.tile_pool(name="stage", bufs=4))
        mstage = ctx.enter_context(tc.tile_pool(name="mstage", bufs=2))
        work = ctx.enter_context(tc.tile_pool(name="work", bufs=2))
        attnp = ctx.enter_context(tc.tile_pool(name="attnp", bufs=3))
        psum = ctx.enter_context(tc.tile_pool(name="psum", bufs=1, space="PSUM"))

        ident16 = consts.tile([P, P], F16)
        make_identity(nc, ident16)

        # ---- mask tiles are streamed during batch 0 (see load_mask_tile):
        # cast int32 -> fp16 bias (mask-1)*60000; masked scores then get
        # -60000 added pre-exp so exp underflows to exactly 0.0
        mask16 = consts.tile([P, n_rt, s], F16)

        def load_mask_tile(rt):
            mstg = mstage.tile([P, s], I32, tag="mstg", bufs=3)
            nc.sync.dma_start(out=mstg, in_=m_d[rt * P : (rt + 1) * P, :])
            nc.gpsimd.tensor_scalar(
                out=mask16[:, rt, :],
                in0=mstg,
                scalar1=-1.0,
                scalar2=60000.0,
                op0=mybir.AluOpType.add,
                op1=mybir.AluOpType.mult,
            )

        def prep_batch(b):
            """Load q,k,v for batch b; build qT16/kT16 (d-on-partitions) + v16."""
            qT = qkv.tile([P, n_dc, s], F16, tag="qT")
            kT = qkv.tile([P, n_dc, s], F16, tag="kT")
            v16 = qkv.tile([P, n_kc, d], F16, tag="v16")
            # k first: row-tile 0's QK needs all of kT but only qT[:, :, 0:P]
            for src, dstT in ((k_d, kT), (q_d, qT)):
                for g in range(n_rt // 2):  # 2 row-tiles per DMA
                    stg = stage.tile([P, 2, d], F32, tag="stg")
                    nc.sync.dma_start(
                        out=stg,
                        in_=src[b, g * 2 * P : (g + 1) * 2 * P, :].rearrange(
                            "(t p) d -> p t d", p=P
                        ),
                    )
                    c16 = stage.tile([P, 2, d], F16, tag="c16")
                    nc.vector.tensor_copy(c16, stg)
                    for t in range(2):
                        rt = g * 2 + t
                        # transpose [128q x 128d] -> [128d x 128q] per d-chunk
                        pprep = psum.tile([P, n_dc, P], F16, tag="ps_t", bufs=2)
                        for dc in range(n_dc):
                            nc.tensor.transpose(
                                pprep[:, dc, :],
                                c16[:, t, dc * P : (dc + 1) * P],
                                ident16,
                            )
                        nc.vector.tensor_copy(
                            dstT[:, :, rt * P : (rt + 1) * P], pprep
                        )
            for g in range(n_kc // 2):
                stg = stage.tile([P, 2, d], F32, tag="stg")
                nc.sync.dma_start(
                    out=stg,
                    in_=v_d[b, g * 2 * P : (g + 1) * 2 * P, :].rearrange(
                        "(t p) d -> p t d", p=P
                    ),
                )
                nc.vector.tensor_copy(v16[:, g * 2 : (g + 1) * 2, :], stg)
            return qT, kT, v16

        for b in range(b_loc):
            qT, kT, v16 = prep_batch(b)
            if b == 0:
                load_mask_tile(0)
                if n_rt > 1:
                    load_mask_tile(1)
            for rt in range(n_rt):
                if b == 0 and rt + 2 < n_rt:
                    load_mask_tile(rt + 2)
                # ---- mask-bias seed + QK^T + exp, pipelined in half-tiles ----
                n_h = max(1, s // 1024)
                H = s // n_h
                p16 = work.tile([P, s], F16, tag="p16", bufs=3)
                rsums = work.tile([P, n_h], F32, tag="rsums")
                for h in range(n_h):
                    ps_h = psum.tile([P, H], F32, tag="ps_s", bufs=2)
                    for cb in range(H // 512):
                        base = h * H + cb * 512
                        nc.tensor.matmul(
                            ps_h[:, cb * 512 : (cb + 1) * 512],
                            lhsT=ident16,
                            rhs=mask16[:, rt, base : base + 512],
                            start=True,
                            stop=False,
                        )
                    for dc in range(n_dc):
                        for cb in range(H // 512):
                            base = h * H + cb * 512
                            nc.tensor.matmul(
                                ps_h[:, cb * 512 : (cb + 1) * 512],
                                lhsT=qT[:, dc, rt * P : (rt + 1) * P],
                                rhs=kT[:, dc, base : base + 512],
                                start=False,
                                stop=(dc == n_dc - 1),
                            )
                    # exp + partial row-sum (ScalarE, fused PSUM eviction)
                    nc.scalar.activation(
                        p16[:, h * H : (h + 1) * H],
                        ps_h,
                        mybir.ActivationFunctionType.Exp,
                        scale=SCALE,
                        accum_out=rsums[:, h : h + 1],
                    )
                inv = work.tile([P, 1], F32, tag="inv")
                if n_h == 2:
                    rowsum = work.tile([P, 1], F32, tag="rowsum")
                    nc.vector.tensor_add(rowsum, rsums[:, 0:1], rsums[:, 1:2])
                else:
                    rowsum = rsums[:, 0:1]
                nc.vector.reciprocal(inv, rowsum)
                # ---- transpose pm16 via identity matmuls ----
                pT16 = work.tile([P, s], F16, tag="pT16", bufs=3)
                for h in range(n_h):
                    ps_t = psum.tile([P, H], F16, tag="ps_t", bufs=2)
                    for c in range(H // P):
                        cc = h * (H // P) + c
                        nc.tensor.transpose(
                            ps_t[:, c * P : (c + 1) * P],
                            p16[:, cc * P : (cc + 1) * P],
                            ident16,
                        )
                    nc.vector.tensor_copy(pT16[:, h * H : (h + 1) * H], ps_t)
                # ---- attn @ v (PE, accumulate over kcol chunks) ----
                ps_o = psum.tile([P, d], F32, tag="ps_o", bufs=2)
                for c in range(n_kc):
                    nc.tensor.matmul(
                        ps_o,
                        lhsT=pT16[:, c * P : (c + 1) * P],
                        rhs=v16[:, c, :],
                        start=(c == 0),
                        stop=(c == n_kc - 1),
                    )
                # ---- attn weights out — normalize h0 on ScalarE, h1 on
                # VectorE (after pT evicts), so the DVE chain never delays
                # the AV matmuls ----
                attn_f32 = attnp.tile([P, s], F32, tag="attn")
                nc.scalar.activation(
                    attn_f32[:, 0:H],
                    p16[:, 0:H],
                    mybir.ActivationFunctionType.Copy,
                    bias=0.0,
                    scale=inv,
                )
                nc.vector.tensor_scalar(
                    out=attn_f32[:, H:s],
                    in0=p16[:, H:s],
                    scalar1=inv,
                    scalar2=None,
                    op0=mybir.AluOpType.mult,
                )
                nc.sync.dma_start(
                    out=a_d[b, rt * P : (rt + 1) * P, :], in_=attn_f32
                )
                # ---- scale by inv (ScalarE) + DMA out ----
                o_f32 = work.tile([P, d], F32, tag="o_f32")
                nc.vector.tensor_scalar(
                    out=o_f32, in0=ps_o, scalar1=inv, scalar2=None,
                    op0=mybir.AluOpType.mult,
                )
                nc.sync.dma_start(out=o_d[b, rt * P : (rt + 1) * P, :], in_=o_f32)

    nc.finalize()
    return nc


_NC_CACHE = {}


def _get_nc():
    if "nc" not in _NC_CACHE:
        _NC_CACHE["nc"] = build_kernel()
    return _NC_CACHE["nc"]


def _ensure_ntff_hook():
    """The container's antenv stub lacks axon_hooks, so boot never registers
    the NTFF profile hook. Inject the module and register it ourselves."""
    import sys as _sys
    import types as _types

    if "antenv.axon_hooks" not in _sys.modules:
        mod = _types.ModuleType("antenv.axon_hooks")
        mod._hook = None

        def set_axon_ntff_profile_hook(h):
            mod._hook = h

        def get_axon_ntff_profile_hook():
            return mod._hook

        mod.set_axon_ntff_profile_hook = set_axon_ntff_profile_hook
        mod.get_axon_ntff_profile_hook = get_axon_ntff_profile_hook
        _sys.modules["antenv.axon_hooks"] = mod
        import antenv

        antenv.axon_hooks = mod
    mod = _sys.modules["antenv.axon_hooks"]
    if mod.get_axon_ntff_profile_hook() is None:
        from trn_agent_boot.trn_boot import _ntff_profile_via_ctypes

        mod.set_axon_ntff_profile_hook(
            _ntff_profile_via_ctypes("/opt/axon/libaxon_pjrt.so")
        )


def kernel(q, k, v, mask, trace=False):
    q = np.ascontiguousarray(np.asarray(q, dtype=np.float32))
    k = np.ascontiguousarray(np.asarray(k, dtype=np.float32))
    v = np.ascontiguousarray(np.asarray(v, dtype=np.float32))
    mask_full = np.ascontiguousarray(np.asarray(mask, dtype=np.int32)).reshape(S, S)

    if trace:
        _ensure_ntff_hook()
    nc = _get_nc()
    in_maps = []
    for c in range(N_CORES):
        sl = slice(c * B_LOC, (c + 1) * B_LOC)
        in_maps.append(
            {"q": q[sl], "k": k[sl], "v": v[sl], "mask": mask_full}
        )
    res = run_bass_kernel_spmd(nc, in_maps, list(range(N_CORES)), trace=trace)
    out = np.concatenate([res.results[c]["out"] for c in range(N_CORES)], axis=0)
    attn = np.concatenate([res.results[c]["attn"] for c in range(N_CORES)], axis=0)
    if trace:
        return (out, attn), res
    return out, attn
